# revision 6
# baseline (speedup 1.0000x reference)
"""BitLinear-1.58 (ternary-weight dense) Trainium2 kernel — fp8 DoubleRow.

Reference computes:
    a  = clip(max(|x|, axis=-1), 1e-5)          [B,S,1]
    out = ((x / a) @ W.T) * (a * ws) + bias
The absmax normalization cancels algebraically -- (x/a)@W * a*ws == x@W * ws
exactly, including the clip (the same clipped `a` divides and multiplies).
So the kernel is a plain matmul + scale + bias:
    out = x @ W.T * ws + bias

Strategy (8 NeuronCores, tensor-parallel along out_features):
  - Each core owns N_C = 11008/8 = 1376 output features (column parallel).
  - fp8 DoubleRow trick: the PE's DoubleRow perf mode contracts 2 fp8
    values per cell per pass (d = w0*m0 + w1*m1) at 0.5 cycles/row --
    2x the bf16 rate.  We split x into fp8e4m3 hi/lo parts
    (x ~= hi + lo to ~8 mantissa bits, bf16-level accuracy) and pair
    each (hi, lo) k-subtile against a DUPLICATED ternary-weight subtile:
        d = hi*w + lo*w = (hi+lo)*w ~= x*w
    One DoubleRow pass == full near-bf16-accuracy matmul at fp8 speed.
  - x^T is the stationary operand; each stationary tile is reused for
    all 3 output n-chunks (3 PSUM banks accumulate concurrently), so the
    256-column DoubleRow LDWEIGHTS is amortized/hidden under ~324ns of
    moving-operand work.
  - Per output tile [128m x 512n]: 32 DoubleRow matmuls accumulate in
    PSUM; a DVE scalar_tensor_tensor applies out = psum * ws + bias;
    DMA to DRAM in the natural [M, N_C] layout.
"""

import numpy as np

import concourse.bass as bass
import concourse.mybir as mybir
import concourse.tile as tile
from concourse import bacc
from concourse.bass_utils import run_bass_kernel_spmd

P = 128
B_DIM, S_DIM, K_DIM, N_FULL = 4, 2048, 4096, 11008
M_DIM = B_DIM * S_DIM            # 8192
N_CORES = 8
N_C = N_FULL // N_CORES          # 1376 per-core output features
KT = K_DIM // P                  # 32 k-blocks of 128
KT2 = 2 * KT                     # 64 subtiles: (hi, lo) interleaved per block
M_BLK = 512                      # m columns per x slab
MT_PER_BLK = M_BLK // P          # stationary tiles per slab
N_CHUNKS = (512, 512, 352)       # moving-operand out-chunks (sum = N_C)


def build_nc(n_repeat=1):
    """n_repeat > 1 re-runs the whole computation that many times inside one
    NEFF (identical output) -- used only for overhead-free timing:
    hw_time = (t[R] - t[1]) / (R - 1)."""
    nc = bacc.Bacc("TRN2", target_bir_lowering=False, debug=False)
    f8, f32 = mybir.dt.float8e4, mybir.dt.float32
    DR = mybir.MatmulPerfMode.DoubleRow

    xt = nc.dram_tensor("xt", [KT2 * P, M_DIM], f8, kind="ExternalInput")
    wt = nc.dram_tensor("wt", [KT2 * P, N_C], f8, kind="ExternalInput")
    bias_rep = nc.dram_tensor("bias_rep", [P, N_C], f32, kind="ExternalInput")
    ws_col = nc.dram_tensor("ws_col", [P, 1], f32, kind="ExternalInput")
    out = nc.dram_tensor("out", [M_DIM, N_C], f32, kind="ExternalOutput")

    xt_v = xt.rearrange("(s p) m -> p s m", p=P)
    wt_v = wt.rearrange("(s p) n -> p s n", p=P)

    n_off = []
    o = 0
    for w in N_CHUNKS:
        n_off.append(o)
        o += w

    with tile.TileContext(nc) as tc:
        with tc.tile_pool(name="const", bufs=1) as const, \
             tc.tile_pool(name="xp", bufs=2) as xp, \
             tc.tile_pool(name="op", bufs=4) as op, \
             tc.tile_pool(name="ps", bufs=2, space="PSUM") as ps:
            # weights fully SBUF-resident: loaded once, reused by all m-blocks
            w_sb = const.tile([P, KT2, N_C], f8)
            nc.sync.dma_start(w_sb[:], wt_v[:])
            bias_sb = const.tile([P, N_C], f32)
            nc.sync.dma_start(bias_sb[:], bias_rep[:])
            ws_sb = const.tile([P, 1], f32)
            nc.sync.dma_start(ws_sb[:], ws_col[:])

            for mb_rep in range(n_repeat * (M_DIM // M_BLK)):
                mb = mb_rep % (M_DIM // M_BLK)
                mo = mb * M_BLK
                xs = xp.tile([P, KT2, M_BLK], f8, tag="x")
                nc.sync.dma_start(xs[:], xt_v[:, :, mo:mo + M_BLK])
                for mt in range(MT_PER_BLK):
                    mtile = slice(mt * P, (mt + 1) * P)
                    pts = [ps.tile([P, 512], f32, name=f"pt{ci}")
                           for ci in range(len(N_CHUNKS))]
                    for t in range(KT):
                        for ci, ncw in enumerate(N_CHUNKS):
                            no = n_off[ci]
                            nc.tensor.matmul(
                                pts[ci][:, :ncw],
                                xs[:, 2 * t:2 * t + 2, mtile],
                                w_sb[:, 2 * t:2 * t + 2, no:no + ncw],
                                start=(t == 0), stop=(t == KT - 1),
                                perf_mode=DR)
                    for ci, ncw in enumerate(N_CHUNKS):
                        no = n_off[ci]
                        ot = op.tile([P, 512], f32, tag="o")
                        nc.vector.scalar_tensor_tensor(
                            ot[:, :ncw], pts[ci][:, :ncw], ws_sb[:, 0:1],
                            bias_sb[:, no:no + ncw],
                            op0=mybir.AluOpType.mult, op1=mybir.AluOpType.add)
                        nc.sync.dma_start(
                            out[mo + mt * P:mo + (mt + 1) * P, no:no + ncw],
                            ot[:, :ncw])

    nc.compile()
    return nc


def _dedupe_ldweights(nc):
    """Legalization splits every InstMatmult into InstLdweights + InstMatmult,
    even when consecutive matmuls share the same stationary operand.  The
    DoubleRow LDWEIGHTS (256 columns, no FWL) costs ~184ns vs ~120ns for the
    N=512 matmul itself, so a per-matmul reload makes the kernel LDW-bound.
    Drop an InstLdweights when the previous one on the queue has an identical
    weights AP and the duplicate carries no sync actions: the matmuls in
    between reuse the already-loaded stationary (valid for non-fp32 dtypes).
    Matmuls/DMAs/semaphores never clobber the PE weight array, so only a
    different InstLdweights invalidates the tracked key."""
    import concourse.mybir as mybir
    dropped = 0
    for fn in nc.m.functions:
        for blk in getattr(fn, "blocks", []) or []:
            insts = blk.instructions
            new = []
            last_key = None
            for inst in insts:
                if isinstance(inst, mybir.InstLdweights):
                    ap = inst.ins[0]
                    key = (ap.memref, ap.offset, str(ap.ap),
                           str(inst.perf_mode), str(inst.is_transpose))
                    si = inst.sync_info
                    clean = si is None or (not si.on_wait and not si.on_update)
                    if key == last_key and clean:
                        dropped += 1
                        continue
                    last_key = key
                new.append(inst)
            if dropped:
                blk.instructions[:] = new
    if dropped:
        print(f"_dedupe_ldweights: dropped {dropped} redundant InstLdweights")


def prep_inputs(x, weight_ternary, weight_scale, bias):
    import ml_dtypes
    f8 = ml_dtypes.float8_e4m3   # TRN FP8_EXP4 flavor (max normal +-240)

    x2d = np.asarray(x, dtype=np.float32).reshape(M_DIM, K_DIM)
    xt = np.ascontiguousarray(x2d.T)                      # [K, M] fp32
    hi = xt.astype(f8)
    lo = (xt - hi.astype(np.float32)).astype(f8)
    # interleave hi/lo k-blocks: subtile 2t = hi block t, 2t+1 = lo block t
    xp = np.empty((KT, 2, P, M_DIM), dtype=f8)
    xp[:, 0] = hi.reshape(KT, P, M_DIM)
    xp[:, 1] = lo.reshape(KT, P, M_DIM)
    xt_pair = np.ascontiguousarray(xp.reshape(KT2 * P, M_DIM))

    ws_col = np.full((P, 1), np.float32(np.asarray(weight_scale).reshape(-1)[0]),
                     dtype=np.float32)
    in_maps = []
    w_all = np.asarray(weight_ternary)
    b_all = np.asarray(bias, dtype=np.float32)
    for c in range(N_CORES):
        rows = slice(c * N_C, (c + 1) * N_C)
        w_c = np.ascontiguousarray(w_all[rows, :].T).astype(np.float32)  # [K, N_C]
        w3 = w_c.reshape(KT, P, N_C)
        wpair = np.empty((KT, 2, P, N_C), dtype=f8)
        wpair[:, 0] = w3.astype(f8)          # ternary: exact in fp8
        wpair[:, 1] = wpair[:, 0]
        wt_c = np.ascontiguousarray(wpair.reshape(KT2 * P, N_C))
        bias_c = np.ascontiguousarray(
            np.broadcast_to(b_all[rows][None, :], (P, N_C)))
        in_maps.append({"xt": xt_pair, "wt": wt_c, "bias_rep": bias_c,
                        "ws_col": ws_col})
    return in_maps


def gather_output(results):
    cols = [results[c]["out"] for c in range(N_CORES)]
    return np.concatenate(cols, axis=1).reshape(B_DIM, S_DIM, N_FULL)


def kernel(x, weight_ternary, weight_scale, bias):
    nc = build_nc()
    in_maps = prep_inputs(x, weight_ternary, weight_scale, bias)
    res = run_bass_kernel_spmd(nc, in_maps, core_ids=list(range(N_CORES)))
    return gather_output(res.results)


if __name__ == "__main__":
    rng = np.random.default_rng(0)
    x = rng.standard_normal((B_DIM, S_DIM, K_DIM)).astype(np.float32)
    w = rng.integers(-1, 2, size=(N_FULL, K_DIM)).astype(np.int8)
    ws = np.full((1,), 0.02, np.float32)
    b = (rng.standard_normal(N_FULL) * 0.01).astype(np.float32)
    out = kernel(x, w, ws, b)
    print(out.shape, out.dtype)


# revision 7
# speedup vs baseline: 1.2009x; 1.2009x over previous
"""BitLinear-1.58 (ternary-weight dense) Trainium2 kernel — fp8 DoubleRow.

Reference computes:
    a  = clip(max(|x|, axis=-1), 1e-5)          [B,S,1]
    out = ((x / a) @ W.T) * (a * ws) + bias
The absmax normalization cancels algebraically -- (x/a)@W * a*ws == x@W * ws
exactly, including the clip (the same clipped `a` divides and multiplies).
So the kernel is a plain matmul + scale + bias:
    out = x @ W.T * ws + bias

Strategy (8 NeuronCores, tensor-parallel along out_features):
  - Each core owns N_C = 11008/8 = 1376 output features (column parallel).
  - fp8 DoubleRow trick: the PE's DoubleRow perf mode contracts 2 fp8
    values per cell per pass (d = w0*m0 + w1*m1) at 0.5 cycles/row --
    2x the bf16 rate.  We split x into fp8e4m3 hi/lo parts
    (x ~= hi + lo to ~8 mantissa bits, bf16-level accuracy) and pair
    each (hi, lo) k-subtile against a DUPLICATED ternary-weight subtile:
        d = hi*w + lo*w = (hi+lo)*w ~= x*w
    One DoubleRow pass == full near-bf16-accuracy matmul at fp8 speed.
  - DoubleRowSwInterleave: the stationary (x^T) is pre-interleaved on the
    host into the hardware's native A/B-pair column-reversed layout
    (flat[2i+j] = M_j[:, 127-i]), making the 256-column LDWEIGHTS read
    contiguous (plain DoubleRow's on-the-fly interleave defeats fast
    weight load and makes LDWEIGHTS the ~184ns critical path vs the
    ~120ns matmul).
  - Each stationary tile is reused by 3 matmuls (the 3 output n-chunks,
    3 PSUM banks accumulating concurrently), maximizing moving work per
    weight load.
  - Per output tile [128m x 512n]: 32 DoubleRow matmuls accumulate in
    PSUM; a DVE scalar_tensor_tensor applies out = psum * ws + bias;
    DMA to DRAM in the natural [M, N_C] layout.
"""

import numpy as np

import concourse.bass as bass
import concourse.mybir as mybir
import concourse.tile as tile
from concourse import bacc
from concourse.bass_utils import run_bass_kernel_spmd

P = 128
B_DIM, S_DIM, K_DIM, N_FULL = 4, 2048, 4096, 11008
M_DIM = B_DIM * S_DIM            # 8192
N_CORES = 8
N_C = N_FULL // N_CORES          # 1376 per-core output features
KT = K_DIM // P                  # 32 k-blocks of 128
KT2 = 2 * KT                     # 64 w-subtiles: (hi, lo) pair per block
MT_TOT = M_DIM // P              # 64 global m-tiles
M_BLK = 512                      # m columns per x slab
MT_PER_BLK = M_BLK // P          # stationary tiles per slab
N_CHUNKS = (512, 512, 352)       # moving-operand out-chunks (sum = N_C)
SWI = True                       # DoubleRowSwInterleave vs plain DoubleRow


def build_nc(n_repeat=1):
    """n_repeat > 1 re-runs the whole computation that many times inside one
    NEFF (identical output) -- used only for overhead-free timing:
    hw_time = (t[R] - t[1]) / (R - 1)."""
    nc = bacc.Bacc("TRN2", target_bir_lowering=False, debug=False)
    f8, f32 = mybir.dt.float8e4, mybir.dt.float32
    PM = (mybir.MatmulPerfMode.DoubleRowSwInterleave if SWI
          else mybir.MatmulPerfMode.DoubleRow)

    if SWI:
        xt = nc.dram_tensor("xt", [KT * P, 2 * M_DIM], f8, kind="ExternalInput")
        xt_v = xt.rearrange("(t p) (mt two m) -> p t mt two m",
                            p=P, two=2, m=P)
    else:
        xt = nc.dram_tensor("xt", [KT2 * P, M_DIM], f8, kind="ExternalInput")
        xt_v = xt.rearrange("(s p) m -> p s m", p=P)
    wt = nc.dram_tensor("wt", [KT2 * P, N_C], f8, kind="ExternalInput")
    bias_rep = nc.dram_tensor("bias_rep", [P, N_C], f32, kind="ExternalInput")
    ws_col = nc.dram_tensor("ws_col", [P, 1], f32, kind="ExternalInput")
    out = nc.dram_tensor("out", [M_DIM, N_C], f32, kind="ExternalOutput")

    wt_v = wt.rearrange("(s p) n -> p s n", p=P)

    n_off = []
    o = 0
    for w in N_CHUNKS:
        n_off.append(o)
        o += w

    with tile.TileContext(nc) as tc:
        with tc.tile_pool(name="const", bufs=1) as const, \
             tc.tile_pool(name="xp", bufs=2) as xp, \
             tc.tile_pool(name="op", bufs=4) as op, \
             tc.tile_pool(name="ps", bufs=2, space="PSUM") as ps:
            # weights fully SBUF-resident: loaded once, reused by all m-blocks
            w_sb = const.tile([P, KT2, N_C], f8)
            nc.sync.dma_start(w_sb[:], wt_v[:])
            bias_sb = const.tile([P, N_C], f32)
            nc.sync.dma_start(bias_sb[:], bias_rep[:])
            ws_sb = const.tile([P, 1], f32)
            nc.sync.dma_start(ws_sb[:], ws_col[:])

            for mb_rep in range(n_repeat * (M_DIM // M_BLK)):
                mb = mb_rep % (M_DIM // M_BLK)
                mo = mb * M_BLK
                if SWI:
                    xs = xp.tile([P, KT, MT_PER_BLK, 2, P], f8, tag="x")
                    nc.sync.dma_start(
                        xs[:],
                        xt_v[:, :, mb * MT_PER_BLK:(mb + 1) * MT_PER_BLK, :, :])
                else:
                    xs = xp.tile([P, KT2, M_BLK], f8, tag="x")
                    nc.sync.dma_start(xs[:], xt_v[:, :, mo:mo + M_BLK])
                for mt in range(MT_PER_BLK):
                    mtile = slice(mt * P, (mt + 1) * P)
                    pts = [ps.tile([P, 512], f32, name=f"pt{ci}")
                           for ci in range(len(N_CHUNKS))]
                    for t in range(KT):
                        stat = (xs[:, t, mt, :, :] if SWI
                                else xs[:, 2 * t:2 * t + 2, mtile])
                        for ci, ncw in enumerate(N_CHUNKS):
                            no = n_off[ci]
                            nc.tensor.matmul(
                                pts[ci][:, :ncw],
                                stat,
                                w_sb[:, 2 * t:2 * t + 2, no:no + ncw],
                                start=(t == 0), stop=(t == KT - 1),
                                perf_mode=PM)
                    for ci, ncw in enumerate(N_CHUNKS):
                        no = n_off[ci]
                        ot = op.tile([P, 512], f32, tag="o")
                        nc.vector.scalar_tensor_tensor(
                            ot[:, :ncw], pts[ci][:, :ncw], ws_sb[:, 0:1],
                            bias_sb[:, no:no + ncw],
                            op0=mybir.AluOpType.mult, op1=mybir.AluOpType.add)
                        nc.sync.dma_start(
                            out[mo + mt * P:mo + (mt + 1) * P, no:no + ncw],
                            ot[:, :ncw])

    nc.compile()
    return nc


def prep_inputs(x, weight_ternary, weight_scale, bias):
    import ml_dtypes
    f8 = ml_dtypes.float8_e4m3   # TRN FP8_EXP4 flavor (max normal +-240)

    x2d = np.asarray(x, dtype=np.float32).reshape(M_DIM, K_DIM)
    xt = np.ascontiguousarray(x2d.T)                      # [K, M] fp32
    hi = xt.astype(f8)
    lo = (xt - hi.astype(np.float32)).astype(f8)
    if SWI:
        # interleave within each 128-m-tile: flat[2i+j] = M_j[:, 127-i]
        hi4 = hi.reshape(KT, P, MT_TOT, P)[..., ::-1]
        lo4 = lo.reshape(KT, P, MT_TOT, P)[..., ::-1]
        sw = np.stack([hi4, lo4], axis=-1)                # [KT,P,MT,128,2]
        xt_pair = np.ascontiguousarray(sw.reshape(KT * P, 2 * M_DIM))
    else:
        # interleave hi/lo k-blocks: subtile 2t = hi block t, 2t+1 = lo
        xp = np.empty((KT, 2, P, M_DIM), dtype=f8)
        xp[:, 0] = hi.reshape(KT, P, M_DIM)
        xp[:, 1] = lo.reshape(KT, P, M_DIM)
        xt_pair = np.ascontiguousarray(xp.reshape(KT2 * P, M_DIM))

    ws_col = np.full((P, 1), np.float32(np.asarray(weight_scale).reshape(-1)[0]),
                     dtype=np.float32)
    in_maps = []
    w_all = np.asarray(weight_ternary)
    b_all = np.asarray(bias, dtype=np.float32)
    for c in range(N_CORES):
        rows = slice(c * N_C, (c + 1) * N_C)
        w_c = np.ascontiguousarray(w_all[rows, :].T).astype(np.float32)  # [K, N_C]
        w3 = w_c.reshape(KT, P, N_C)
        wpair = np.empty((KT, 2, P, N_C), dtype=f8)
        wpair[:, 0] = w3.astype(f8)          # ternary: exact in fp8
        wpair[:, 1] = wpair[:, 0]
        wt_c = np.ascontiguousarray(wpair.reshape(KT2 * P, N_C))
        bias_c = np.ascontiguousarray(
            np.broadcast_to(b_all[rows][None, :], (P, N_C)))
        in_maps.append({"xt": xt_pair, "wt": wt_c, "bias_rep": bias_c,
                        "ws_col": ws_col})
    return in_maps


def gather_output(results):
    cols = [results[c]["out"] for c in range(N_CORES)]
    return np.concatenate(cols, axis=1).reshape(B_DIM, S_DIM, N_FULL)


def kernel(x, weight_ternary, weight_scale, bias):
    nc = build_nc()
    in_maps = prep_inputs(x, weight_ternary, weight_scale, bias)
    res = run_bass_kernel_spmd(nc, in_maps, core_ids=list(range(N_CORES)))
    return gather_output(res.results)


if __name__ == "__main__":
    rng = np.random.default_rng(0)
    x = rng.standard_normal((B_DIM, S_DIM, K_DIM)).astype(np.float32)
    w = rng.integers(-1, 2, size=(N_FULL, K_DIM)).astype(np.int8)
    ws = np.full((1,), 0.02, np.float32)
    b = (rng.standard_normal(N_FULL) * 0.01).astype(np.float32)
    out = kernel(x, w, ws, b)
    print(out.shape, out.dtype)
